# revision 1
# baseline (speedup 1.0000x reference)
"""Correlation layer (FlowNet-style) Trainium2 Bass kernel.

Problem: in1, in2: [8, 256, 128, 128] fp32.
out[b, 9*dy+dx, y, x] = mean_c in1[b,c,y,x] * in2pad[b,c,y+dy,x+dx],
with in2 zero-padded by 4 on each spatial side, dy,dx in [0,9).
Output: [8, 81, 128, 128] fp32.

Sharding: data-parallel over batch -> 8 NeuronCores, one batch each
(SPMD: identical program, per-core input slices).

Per-core algorithm:
  Phase 1 (Gram matmuls), tiles of 128 output pixels (y-block 32 x x-block 4):
      stationary = in1[c, ytile, xtile]  (128 cols, x-outer/y-inner:
                                          i = x_off*32 + y_off)
      moving     = in2pad[c, y0:y0+40, x0:x0+12]  (480 cols, fp32r full rate)
      psum[i, j] = sum_c stat[c,i] * mov[c,j]  (accumulated over 2 c-blocks)
    The 81 correlation outputs of pixel i sit at j = (y_off+dy)*12+(x_off+dx),
    a sheared band.  Evacuate psum -> SBUF with *1/256; window-compact per
    32-partition group g (all pixels of a group share x_off=g, so the 9-col
    window [g, g+9) is partition-uniform; engine APs must start at partition
    0/32/64/96 which a 32-group satisfies).  In the compacted [40, 9] block
    the 81 useful values of pixel (g, u) are rows [u, u+9) = one contiguous
    324-byte run.  Dump to DRAM scratch.
  Phase 2: per-group DMA gather (flat DRAM addressing absorbs the
    partition-dependent run offset 9u), TensorE transpose
    [pixel, 81] -> [81, pixel], evacuate with the (x-outer,y-inner) -> (y,x)
    reorder, store [81, y, x] row-blocks.
"""

import numpy as np
from contextlib import ExitStack

import concourse.bacc as bacc
import concourse.tile as tile
import concourse.mybir as mybir
import concourse.bass as bass
from concourse import bass_utils

# ---- problem constants (hardcoded per contract) ----
B = 8
C = 256
H = W = 128
PAD = 4
D = 9            # displacements per axis
CH = D * D       # 81 output channels
HP = WP = H + 2 * PAD   # 136 padded

YB = 32          # y rows per tile
XBW = 4          # x cols per tile (stationary width)
MV_Y = YB + 8    # moving window rows   (40)
MV_X = XBW + 8   # moving window cols   (12)
N_YB = H // YB   # 4
N_XB = W // XBW  # 32
N_TILES = N_YB * N_XB   # 128
PSUM_F = MV_Y * MV_X    # 480 moving cols per tile
NG = 128 // YB   # 4 groups of 32 partitions per tile

# in2pad is held in SBUF as two y-halves (full padded tensor would not fit)
HALF_ROWS = 72   # padded rows per half: [0,72) and [64,136)

FP32 = mybir.dt.float32
FP32R = mybir.dt.float32r

USE_WINDOWS = True


def prep_in1(in1_b: np.ndarray) -> np.ndarray:
    """[C, H, W] -> [C, yb, x, y32]: makes each tile's stationary operand a
    contiguous 128-column slice (walrus requires single-free-dim weights)."""
    return np.ascontiguousarray(
        in1_b.reshape(C, N_YB, YB, W).swapaxes(2, 3)
    )


def build_nc():
    nc = bacc.Bacc("TRN2", target_bir_lowering=False, debug=False)
    in1_d = nc.dram_tensor("in1", [C, N_YB, W, YB], FP32, kind="ExternalInput").ap()
    in2_d = nc.dram_tensor("in2", [C, H, W], FP32, kind="ExternalInput").ap()
    out_d = nc.dram_tensor("out", [CH, H, W], FP32, kind="ExternalOutput").ap()
    if USE_WINDOWS:
        sdump_t = nc.dram_tensor("sdump", [N_TILES, 128, MV_Y, D], FP32, kind="Internal")
    else:
        sdump_t = nc.dram_tensor("sdump", [N_TILES, 128, MV_Y, MV_X], FP32, kind="Internal")
    sdump = sdump_t.ap()

    with tile.TileContext(nc) as tc, ExitStack() as es:
        const_pool = es.enter_context(tc.tile_pool(name="const", bufs=1))
        in2_pool = es.enter_context(tc.tile_pool(name="in2p", bufs=1))
        in1_pool = es.enter_context(tc.tile_pool(name="in1c", bufs=2))
        s_pool = es.enter_context(tc.tile_pool(name="sevac", bufs=3))
        w_pool = es.enter_context(tc.tile_pool(name="wcomp", bufs=4))
        t_pool = es.enter_context(tc.tile_pool(name="tgath", bufs=4))
        o_pool = es.enter_context(tc.tile_pool(name="oasm", bufs=3))
        psum_pool = es.enter_context(tc.tile_pool(name="psum", bufs=4, space="PSUM"))
        psum2_pool = es.enter_context(tc.tile_pool(name="psum2", bufs=2, space="PSUM"))

        # ---- identity matrix for TensorE transpose ----
        ones = const_pool.tile([128, 128], FP32, tag="ones")
        ident = const_pool.tile([128, 128], FP32, tag="ident")
        nc.gpsimd.memset(ones[:, :], 1.0)
        # iota[p, f] = f - p; ident = where(iota == 0, ones, 0)
        nc.gpsimd.affine_select(
            ident[:, :], ones[:, :], pattern=[[1, 128]],
            compare_op=mybir.AluOpType.is_equal, fill=0.0,
            base=0, channel_multiplier=-1,
        )

        # =========================== phase 1 ===========================
        for half in range(2):
            # padded rows [row0, row0+72) of in2pad live in SBUF this pass
            row0 = 0 if half == 0 else HP - HALF_ROWS  # 0 or 64
            in2p = in2_pool.tile([128, 2, HALF_ROWS, WP], FP32R, tag="in2p")
            # interior <- in2 rows [row0-4, row0+68-4) clipped to [0, 128)
            src_lo = max(row0 - PAD, 0)              # 0 / 60
            src_hi = min(row0 + HALF_ROWS - PAD, H)  # 68 / 128
            dst_lo = src_lo + PAD - row0             # 4 / 0
            dst_hi = dst_lo + (src_hi - src_lo)      # 72?no: 4+68=72 -> trimmed below
            # top/bottom zero rows within this half
            if dst_lo > 0:
                nc.vector.memset(in2p[:, :, 0:dst_lo, :].bitcast(FP32), 0.0)
            if dst_hi < HALF_ROWS:
                nc.vector.memset(in2p[:, :, dst_hi:HALF_ROWS, :].bitcast(FP32), 0.0)
            nc.gpsimd.memset(in2p[:, :, dst_lo:dst_hi, 0:PAD].bitcast(FP32), 0.0)
            nc.gpsimd.memset(in2p[:, :, dst_lo:dst_hi, WP - PAD:WP].bitcast(FP32), 0.0)
            for cb in range(2):
                nc.sync.dma_start(
                    in2p[:, cb, dst_lo:dst_hi, PAD:PAD + W],
                    in2_d[cb * 128:(cb + 1) * 128, src_lo:src_hi, :].bitcast(FP32R),
                )

            for yb in (0 + 2 * half, 1 + 2 * half):
                y0 = yb * YB             # global padded row of window start
                y0l = y0 - row0          # row within this half's SBUF tile
                in1c = in1_pool.tile([128, 2, W, YB], FP32R, tag="in1c")
                for cb in range(2):
                    nc.sync.dma_start(
                        in1c[:, cb, :, :],
                        in1_d[cb * 128:(cb + 1) * 128, yb, :, :].bitcast(FP32R),
                    )
                for xb in range(N_XB):
                    x0 = xb * XBW
                    t = yb * N_XB + xb
                    ps = psum_pool.tile([128, MV_Y, MV_X], FP32, tag="ps")
                    for cb in range(2):
                        stat = in1c[:, cb, x0:x0 + XBW, :].rearrange(
                            "p a b -> p (a b)"
                        )
                        mov = in2p[:, cb, y0l:y0l + MV_Y, x0:x0 + MV_X]
                        nc.tensor.matmul(
                            ps[:, :, :],
                            stat,
                            mov,
                            start=(cb == 0),
                            stop=(cb == 1),
                        )
                    # evacuate + scale (mean over C=256)
                    sv = s_pool.tile([128, MV_Y, MV_X], FP32, tag="sevac")
                    if t % 2 == 0:
                        nc.scalar.mul(sv[:, :, :], ps[:, :, :], 1.0 / C)
                    else:
                        nc.vector.tensor_scalar_mul(sv[:, :, :], ps[:, :, :], 1.0 / C)

                    if USE_WINDOWS:
                        wv = w_pool.tile([128, MV_Y, D], FP32, tag="wcomp")
                        for g in range(NG):
                            src = sv[32 * g:32 * (g + 1), :, g:g + D]
                            dst = wv[32 * g:32 * (g + 1), :, :]
                            e = (t + g) % 4
                            if e == 0:
                                nc.gpsimd.tensor_copy(dst, src)
                            elif e == 1:
                                nc.scalar.copy(dst, src)
                            else:
                                nc.vector.tensor_copy(dst, src)
                        nc.sync.dma_start(sdump[t], wv[:, :, :])
                    else:
                        nc.sync.dma_start(sdump[t], sv[:, :, :])

        # =========================== phase 2 ===========================
        for yb in range(N_YB):
            y0 = yb * YB
            oasm0 = o_pool.tile([128, YB // 2, W], FP32, tag="oasm")
            oasm1 = o_pool.tile([128, YB // 2, W], FP32, tag="oasm")
            oasm = [oasm0, oasm1]
            for xb in range(N_XB):
                x0 = xb * XBW
                t = yb * N_XB + xb
                tg = t_pool.tile([128, CH], FP32, tag="tgath")
                # gather the 81-value run of each pixel (flat DRAM addressing
                # absorbs the partition-dependent shear)
                for g in range(NG):
                    if USE_WINDOWS:
                        # elem offset for (u, k): (t*128 + 32g + u)*360 + 9u + k
                        base = (t * 128 + 32 * g) * (MV_Y * D)
                        src = bass.AP(sdump_t, base, [[MV_Y * D + D, 32], [1, CH]])
                        dst = tg[32 * g:32 * (g + 1), :]
                    else:
                        # elem offset (u, dy, dx):
                        #   (t*128 + 32g + u)*480 + (u+dy)*12 + (g+dx)
                        base = (t * 128 + 32 * g) * PSUM_F + g
                        src = bass.AP(
                            sdump_t, base,
                            [[PSUM_F + MV_X, 32], [MV_X, D], [1, D]],
                        )
                        dst = tg[32 * g:32 * (g + 1), :].rearrange(
                            "p (a b) -> p a b", a=D
                        )
                    nc.sync.dma_start(dst, src)
                # transpose [pixel, 81] -> [81, pixel]
                ps2 = psum2_pool.tile([128, XBW, YB], FP32, tag="ps2")
                nc.tensor.transpose(ps2[0:CH, :, :], tg[:, :], ident[:, :])
                # evacuate with (x-outer, y-inner) -> (y, x) reorder, y-halves
                for hf in range(2):
                    dst = oasm[hf][0:CH, :, x0:x0 + XBW].transpose([0, 2, 1])
                    src = ps2[0:CH, :, 16 * hf:16 * (hf + 1)]
                    if xb % 2 == 0:
                        nc.vector.tensor_copy(dst, src)
                    else:
                        nc.scalar.copy(dst, src)
            for hf in range(2):
                nc.sync.dma_start(
                    out_d[:, y0 + 16 * hf:y0 + 16 * (hf + 1), :],
                    oasm[hf][0:CH, :, :],
                )

    nc.compile()
    return nc


_NC_CACHE = None


def _get_nc():
    global _NC_CACHE
    if _NC_CACHE is None:
        _NC_CACHE = build_nc()
    return _NC_CACHE


def kernel(in1: np.ndarray, in2: np.ndarray) -> np.ndarray:
    nc = _get_nc()
    in1 = np.ascontiguousarray(np.asarray(in1, dtype=np.float32))
    in2 = np.ascontiguousarray(np.asarray(in2, dtype=np.float32))
    assert in1.shape == (B, C, H, W) and in2.shape == (B, C, H, W)
    in_maps = [{"in1": prep_in1(in1[b]), "in2": in2[b]} for b in range(B)]
    res = bass_utils.run_bass_kernel_spmd(nc, in_maps, core_ids=list(range(B)))
    out = np.stack([res.results[b]["out"] for b in range(B)], axis=0)
    return out



# revision 2
# speedup vs baseline: 3.7721x; 3.7721x over previous
"""Correlation layer (FlowNet-style) Trainium2 Bass kernel, v2.

Problem: in1, in2: [8, 256, 128, 128] fp32.
out[b, 9*dy+dx, y, x] = mean_c in1[b,c,y,x] * in2pad[b,c,y+dy,x+dx],
in2 zero-padded by 4 per spatial side, dy,dx in [0,9).  Output
[8, 81, 128, 128] fp32.  Data-parallel over batch: 1 batch / core.

Host prep (free): in1 scaled by 1/256 (folds the channel mean),
(x-outer, y-inner) tile layout, bf16; in2 zero-padded to 136x136, bf16.
Output produced in bf16 and upcast on host.

Per-core pipeline (all bf16 matmul operands, fp32 PSUM accumulate):

1. Correlation matmuls.  Pixel tile = 32 y  x 4 x (128 pixels); each of
   the 4 x-columns is an independent col-tiled matmul (tile_position
   (0,32g)): stationary = in1[c, 32 pixels of column g], moving = its
   own 9-wide window in2pad[c, y0:y0+40, xg:xg+9] (N=360).  The four
   groups run concurrently on the PE array quarters, so a tile costs
   ~2x360 cycles for 128 pixels instead of 2x480 with a shared window,
   and PSUM comes out as [128, 40, 9] with IDENTICAL free layout for
   every partition: pixel (g,u) has channel ch at free offset 9u + ch.

2. Evacuate psum -> SBUF in one full-width copy per tile (cast to
   bf16).  No window-compaction instructions needed.

3. Sheared dump.  DMA the [40,9] blocks to a DRAM scratch where chunk
   of pixel p' (global raster index y*128+x) starts at byte offset
   2*(369*p' - 9u).  The -9u per-partition shear is absorbed by the
   flat DRAM stride (u-stride 369*128-9): each pixel's 81 useful
   channel values land EXACTLY at [369*p', 369*p'+81), and chunks
   never overlap (gap 9 between x-neighbours, exact tiling in y).

4. XBAR transpose read-back: dma_start(transpose=True) with source AP
   [[369, 2048], [1, 128]] reads each pixel's 81 channels (+47 junk
   cols) and transposes to SBUF [128ch, 2048pix] -- already in final
   [channel, raster(y,x)] order.

5. Store rows 0..81 to out (bf16), host upcasts to fp32.
"""

import numpy as np
import ml_dtypes
from contextlib import ExitStack

import concourse.bacc as bacc
import concourse.tile as tile
import concourse.mybir as mybir
import concourse.bass as bass
from concourse import bass_utils

# ---- problem constants (hardcoded per contract) ----
B = 8
C = 256
H = W = 128
PAD = 4
D = 9            # displacements per axis
CH = D * D       # 81 output channels
HP = WP = H + 2 * PAD   # 136 padded

YB = 32          # y rows per pixel tile
XBW = 4          # x cols per pixel tile (one col-tiled matmul each)
MV_Y = YB + 8    # moving window rows per group (40)
N_YB = H // YB   # 4
N_XB = W // XBW  # 32
NG = 4           # col-tile groups per tile
TBATCH = 16      # tiles buffered per dump batch (half a yb row)

BLK = MV_Y * D   # 360 elems per pixel chunk
PITCH = BLK + D  # 369: scratch pitch per pixel
NPIX = H * W     # 16384
SCR_ELEMS = PITCH * NPIX

XH = 2048        # pixels per xbar batch (16 y rows)
N_XBATCH = NPIX // XH  # 8

BF16 = mybir.dt.bfloat16
FP32 = mybir.dt.float32


def prep_in_maps(in1: np.ndarray, in2: np.ndarray) -> list[dict]:
    """Host-side prep: scale+layout in1, pad in2, cast bf16."""
    in1 = np.asarray(in1, dtype=np.float32)
    in2 = np.asarray(in2, dtype=np.float32)
    assert in1.shape == (B, C, H, W) and in2.shape == (B, C, H, W)
    # [B, cb, c, yb, x, y],  scaled by 1/C (folds the channel mean)
    a = (in1 * (1.0 / C)).reshape(B, 2, 128, N_YB, YB, W)
    a = np.ascontiguousarray(a.transpose(0, 1, 2, 3, 5, 4)).astype(
        ml_dtypes.bfloat16
    )
    # [B, cb, c, 136, 136] zero-padded
    p = np.pad(in2, ((0, 0), (0, 0), (PAD, PAD), (PAD, PAD))).reshape(
        B, 2, 128, HP, WP
    ).astype(ml_dtypes.bfloat16)
    return [{"in1": a[b], "in2": p[b]} for b in range(B)]


def build_nc():
    nc = bacc.Bacc("TRN2", target_bir_lowering=False, debug=False)
    in1_d = nc.dram_tensor(
        "in1", [2, 128, N_YB, W, YB], BF16, kind="ExternalInput"
    ).ap()
    in2_d = nc.dram_tensor(
        "in2", [2, 128, HP, WP], BF16, kind="ExternalInput"
    ).ap()
    out_d = nc.dram_tensor("out", [CH, H, W], BF16, kind="ExternalOutput").ap()
    scr_t = nc.dram_tensor("scr", [SCR_ELEMS], BF16, kind="Internal")

    with tile.TileContext(nc) as tc, ExitStack() as es:
        in2_pool = es.enter_context(tc.tile_pool(name="in2p", bufs=1))
        in1_pool = es.enter_context(tc.tile_pool(name="in1c", bufs=2))
        wv_pool = es.enter_context(tc.tile_pool(name="wv", bufs=2))
        xb_pool = es.enter_context(tc.tile_pool(name="xb", bufs=2))
        psum_pool = es.enter_context(tc.tile_pool(name="ps", bufs=4, space="PSUM"))

        # ---- in2 (padded, bf16) resident in SBUF; row-chunked loads so
        # yb0 matmuls start after ~1/4 of the transfer ----
        in2p = in2_pool.tile([128, 2, HP, WP], BF16, tag="in2p")
        row_chunks = [(0, 40), (40, 72), (72, 104), (104, HP)]
        for cb in range(2):
            for r0, r1 in row_chunks:
                nc.sync.dma_start(
                    in2p[:, cb, r0:r1, :],
                    in2_d[cb, :, r0:r1, :],
                )

        for yb in range(N_YB):
            y0 = yb * YB
            in1c = in1_pool.tile([128, 2, W, YB], BF16, tag="in1c")
            for cb in range(2):
                nc.sync.dma_start(in1c[:, cb, :, :], in1_d[cb, :, yb, :, :])

            for half in range(2):
                xbase = half * TBATCH
                wv = wv_pool.tile([128, TBATCH, MV_Y, D], BF16, tag="wv")
                for t in range(TBATCH):
                    xb = xbase + t
                    ps = psum_pool.tile([128, MV_Y, D], FP32, tag="ps")
                    for cb in range(2):
                        for g in range(NG):
                            stat = in1c[:, cb, xb * XBW + g, :]
                            xg = xb * XBW + g
                            mov = in2p[:, cb, y0:y0 + MV_Y, xg:xg + D]
                            nc.tensor.matmul(
                                ps[32 * g:32 * (g + 1), :, :],
                                stat,
                                mov,
                                start=(cb == 0),
                                stop=(cb == 1),
                                tile_position=(0, 32 * g),
                            )
                    dst = wv[:, t, :, :]
                    if xb % 2 == 0:
                        nc.vector.tensor_copy(dst, ps[:, :, :])
                    else:
                        nc.scalar.copy(dst, ps[:, :, :])

                # sheared dump: pixel (g,u) of tile (yb, xb=xbase+t) ->
                # chunk at 369*p' - 9u, p' = (yb*32+u)*128 + xb*4 + g
                for g in range(NG):
                    src = wv[32 * g:32 * (g + 1), :, :, :].rearrange(
                        "p t a b -> p t (a b)"
                    )
                    base = PITCH * (yb * YB * W + XBW * xbase + g)
                    dst = bass.AP(
                        scr_t,
                        base,
                        [[PITCH * W - D, 32], [PITCH * XBW, TBATCH], [1, BLK]],
                    )
                    nc.sync.dma_start(dst, src)

            # ---- read back via xbar transpose + store; the two xbar
            # batches of this yb need both dumps above (DRAM deps) ----
            for k in (2 * yb, 2 * yb + 1):
                xbt = xb_pool.tile([128, XH], BF16, tag="xbt")
                src = bass.AP(scr_t, PITCH * XH * k, [[PITCH, XH], [1, 128]])
                nc.sync.dma_start(xbt[:, :], src, transpose=True)
                nc.sync.dma_start(
                    out_d[:, 16 * k:16 * (k + 1), :].rearrange(
                        "c a b -> c (a b)"
                    ),
                    xbt[0:CH, :],
                )

    nc.compile()
    return nc


_NC_CACHE = None


def _get_nc():
    global _NC_CACHE
    if _NC_CACHE is None:
        _NC_CACHE = build_nc()
    return _NC_CACHE


def kernel(in1: np.ndarray, in2: np.ndarray) -> np.ndarray:
    nc = _get_nc()
    in_maps = prep_in_maps(in1, in2)
    res = bass_utils.run_bass_kernel_spmd(nc, in_maps, core_ids=list(range(B)))
    return np.stack(
        [res.results[b]["out"].astype(np.float32) for b in range(B)], axis=0
    )


# revision 3
# speedup vs baseline: 4.2485x; 1.1263x over previous
"""Correlation layer (FlowNet-style) Trainium2 Bass kernel, v2.

Problem: in1, in2: [8, 256, 128, 128] fp32.
out[b, 9*dy+dx, y, x] = mean_c in1[b,c,y,x] * in2pad[b,c,y+dy,x+dx],
in2 zero-padded by 4 per spatial side, dy,dx in [0,9).  Output
[8, 81, 128, 128] fp32.  Data-parallel over batch: 1 batch / core.

Host prep (free): in1 scaled by 1/256 (folds the channel mean),
(x-outer, y-inner) tile layout, bf16; in2 zero-padded to 136x136, bf16.
Output produced in bf16 and upcast on host.

Per-core pipeline (all bf16 matmul operands, fp32 PSUM accumulate):

1. Correlation matmuls.  Pixel tile = 32 y  x 4 x (128 pixels); each of
   the 4 x-columns is an independent col-tiled matmul (tile_position
   (0,32g)): stationary = in1[c, 32 pixels of column g], moving = its
   own 9-wide window in2pad[c, y0:y0+40, xg:xg+9] (N=360).  The four
   groups run concurrently on the PE array quarters, so a tile costs
   ~2x360 cycles for 128 pixels instead of 2x480 with a shared window,
   and PSUM comes out as [128, 40, 9] with IDENTICAL free layout for
   every partition: pixel (g,u) has channel ch at free offset 9u + ch.

2. Evacuate psum -> SBUF in one full-width copy per tile (cast to
   bf16).  No window-compaction instructions needed.

3. Sheared dump.  DMA the [40,9] blocks to a DRAM scratch where chunk
   of pixel p' (global raster index y*128+x) starts at byte offset
   2*(369*p' - 9u).  The -9u per-partition shear is absorbed by the
   flat DRAM stride (u-stride 369*128-9): each pixel's 81 useful
   channel values land EXACTLY at [369*p', 369*p'+81), and chunks
   never overlap (gap 9 between x-neighbours, exact tiling in y).

4. XBAR transpose read-back: dma_start(transpose=True) with source AP
   [[369, 2048], [1, 128]] reads each pixel's 81 channels (+47 junk
   cols) and transposes to SBUF [128ch, 2048pix] -- already in final
   [channel, raster(y,x)] order.

5. Store rows 0..81 to out (bf16), host upcasts to fp32.
"""

import numpy as np
import ml_dtypes
from contextlib import ExitStack

import concourse.bacc as bacc
import concourse.tile as tile
import concourse.mybir as mybir
import concourse.bass as bass
from concourse import bass_utils

# ---- problem constants (hardcoded per contract) ----
B = 8
C = 256
H = W = 128
PAD = 4
D = 9            # displacements per axis
CH = D * D       # 81 output channels
HP = WP = H + 2 * PAD   # 136 padded

YB = 32          # y rows per pixel tile
XBW = 4          # x cols per pixel tile (one col-tiled matmul each)
MV_Y = YB + 8    # moving window rows per group (40)
N_YB = H // YB   # 4
N_XB = W // XBW  # 32
NG = 4           # col-tile groups per tile
TBATCH = 16      # tiles buffered per dump batch (half a yb row)

BLK = MV_Y * D   # 360 elems per pixel chunk
PITCH = BLK + D  # 369: scratch pitch per pixel
NPIX = H * W     # 16384
SCR_ELEMS = PITCH * NPIX

XH = 2048        # pixels per xbar batch (16 y rows)
N_XBATCH = NPIX // XH  # 8

BF16 = mybir.dt.bfloat16
FP32 = mybir.dt.float32


def prep_in_maps(in1: np.ndarray, in2: np.ndarray) -> list[dict]:
    """Host-side prep: scale+layout in1, pad in2, cast bf16."""
    in1 = np.asarray(in1, dtype=np.float32)
    in2 = np.asarray(in2, dtype=np.float32)
    assert in1.shape == (B, C, H, W) and in2.shape == (B, C, H, W)
    # [B, cb, c, yb, x, y],  scaled by 1/C (folds the channel mean)
    a = (in1 * (1.0 / C)).reshape(B, 2, 128, N_YB, YB, W)
    a = np.ascontiguousarray(a.transpose(0, 1, 2, 3, 5, 4)).astype(
        ml_dtypes.bfloat16
    )
    # [B, cb, c, 136, 136] zero-padded
    p = np.pad(in2, ((0, 0), (0, 0), (PAD, PAD), (PAD, PAD))).reshape(
        B, 2, 128, HP, WP
    ).astype(ml_dtypes.bfloat16)
    return [{"in1": a[b], "in2": p[b]} for b in range(B)]


def build_nc():
    nc = bacc.Bacc("TRN2", target_bir_lowering=False, debug=False)
    in1_d = nc.dram_tensor(
        "in1", [2, 128, N_YB, W, YB], BF16, kind="ExternalInput"
    ).ap()
    in2_d = nc.dram_tensor(
        "in2", [2, 128, HP, WP], BF16, kind="ExternalInput"
    ).ap()
    out_d = nc.dram_tensor("out", [CH, H, W], BF16, kind="ExternalOutput").ap()
    scr_t = nc.dram_tensor("scr", [SCR_ELEMS], BF16, kind="Internal")

    with tile.TileContext(nc) as tc, ExitStack() as es:
        in2_pool = es.enter_context(tc.tile_pool(name="in2p", bufs=1))
        in1_pool = es.enter_context(tc.tile_pool(name="in1c", bufs=1))
        wv_pool = es.enter_context(tc.tile_pool(name="wv", bufs=2))
        xb_pool = es.enter_context(tc.tile_pool(name="xb", bufs=2))
        psum_pool = es.enter_context(tc.tile_pool(name="ps", bufs=4, space="PSUM"))

        # ---- inputs: all loads issue on the scalar (ACT) HWDGE queue so
        # they never queue behind dump/xbar traffic (which lives on the
        # sync queue).  Issue order = first-use order, so the first
        # matmul waits only for ~2.4 MB, and later chunks stream in
        # behind while compute runs. ----
        in2p = in2_pool.tile([128, 2, HP, WP], BF16, tag="in2p")
        in1c = in1_pool.tile([128, 2, N_YB, W, YB], BF16, tag="in1c")
        row_chunks = [(0, 40), (40, 72), (72, 104), (104, HP)]
        for yb in range(N_YB):
            r0, r1 = row_chunks[yb]
            for cb in range(2):
                nc.scalar.dma_start(
                    in1c[:, cb, yb, :, :], in1_d[cb, :, yb, :, :]
                )
                nc.scalar.dma_start(
                    in2p[:, cb, r0:r1, :], in2_d[cb, :, r0:r1, :]
                )

        for yb in range(N_YB):
            y0 = yb * YB
            wv = wv_pool.tile([128, N_XB, MV_Y, D], BF16, tag="wv")
            for xb in range(N_XB):
                ps = psum_pool.tile([128, MV_Y, D], FP32, tag="ps")
                for cb in range(2):
                    for g in range(NG):
                        stat = in1c[:, cb, yb, xb * XBW + g, :]
                        xg = xb * XBW + g
                        mov = in2p[:, cb, y0:y0 + MV_Y, xg:xg + D]
                        nc.tensor.matmul(
                            ps[32 * g:32 * (g + 1), :, :],
                            stat,
                            mov,
                            start=(cb == 0),
                            stop=(cb == 1),
                            tile_position=(0, 32 * g),
                        )
                dst = wv[:, xb, :, :]
                if xb % 2 == 0:
                    nc.vector.tensor_copy(dst, ps[:, :, :])
                else:
                    nc.scalar.copy(dst, ps[:, :, :])

            # sheared dump (sync queue): pixel (g,u) of tile (yb, xb) ->
            # chunk at 369*p' - 9u, p' = (yb*32+u)*128 + xb*4 + g
            for g in range(NG):
                src = wv[32 * g:32 * (g + 1), :, :, :].rearrange(
                    "p t a b -> p t (a b)"
                )
                base = PITCH * (yb * YB * W + g)
                dst = bass.AP(
                    scr_t,
                    base,
                    [[PITCH * W - D, 32], [PITCH * XBW, N_XB], [1, BLK]],
                )
                nc.sync.dma_start(dst, src)

            # ---- read back via xbar transpose (sync, FIFO after the
            # dumps) + store on the gpsimd SWDGE queue ----
            for k in (2 * yb, 2 * yb + 1):
                xbt = xb_pool.tile([128, XH], BF16, tag="xbt")
                src = bass.AP(scr_t, PITCH * XH * k, [[PITCH, XH], [1, 128]])
                nc.sync.dma_start(xbt[:, :], src, transpose=True)
                nc.gpsimd.dma_start(
                    out_d[:, 16 * k:16 * (k + 1), :].rearrange(
                        "c a b -> c (a b)"
                    ),
                    xbt[0:CH, :],
                )

    nc.compile()
    return nc


_NC_CACHE = None


def _get_nc():
    global _NC_CACHE
    if _NC_CACHE is None:
        _NC_CACHE = build_nc()
    return _NC_CACHE


def kernel(in1: np.ndarray, in2: np.ndarray) -> np.ndarray:
    nc = _get_nc()
    in_maps = prep_in_maps(in1, in2)
    res = bass_utils.run_bass_kernel_spmd(nc, in_maps, core_ids=list(range(B)))
    return np.stack(
        [res.results[b]["out"].astype(np.float32) for b in range(B)], axis=0
    )
